# revision 1
# baseline (speedup 1.0000x reference)
"""PlatonicConv (graph-mode attention) Trainium2 Bass kernel.

Math (per graph of 64 fully-connected nodes, 24 group-heads of dim 16):
  q/k/v = x @ W; RoPE(q, k) from pos; S = q.k^T/4; softmax over dst;
  out = A @ v; y = out @ Wo.  32 graphs -> data-parallel over 8 cores.

Key layout choices (per core: 4 graphs, 256 nodes):
  * Everything attention-side lives transposed ([feature, node]) so the
    per-head score matmuls need no activation transposes.
  * Heads are "spread" to 32-aligned partition slots so score matmuls
    pack 4-way into the PE array via tile_position row groups.
  * Softmax is max-free (scores are O(1) by construction); the denominator
    comes for free as a 17th row of each AV matmul via an interleaved
    ones-column in the V weights.
"""

import numpy as np

G = 12
H = 2
D = 16
GH = 24          # G * H group-heads
C = 384          # in/emb/out channels
NG = 32          # graphs
NPG = 64         # nodes per graph
N = NG * NPG
NCORES = 8
GPC = NG // NCORES   # graphs per core = 4
NPC = GPC * NPG      # nodes per core = 256
VW = 17              # V block width (16 + ones col)
CAUG = GH * VW       # 408

_F32R_PROJ = True    # big projections in float32r (4x faster PE)

_CACHE = {}


def _host_prep(Wq, Wk, Wv, Wo, rope_freqs):
    f32 = np.float32
    idx = np.arange(C)
    d16 = idx % 16
    partner = np.where(d16 % 2 == 0, idx + 1, idx - 1)
    sign = np.where(d16 % 2 == 0, -1.0, 1.0).astype(f32)
    # pair-swap-negated projections: QpT = Wqp^T X^T, Qp^T[e] = sign(e) Q^T[partner(e)]
    Wqp = (Wq[:, partner] * sign[None, :]).astype(f32)
    Wkp = (Wk[:, partner] * sign[None, :]).astype(f32)

    # V interleaved with a ones column per head: block j = [Wv head j | 0]
    Wvil = np.zeros((C, CAUG), f32)
    for j in range(GH):
        Wvil[:, VW * j:VW * j + 16] = Wv[:, 16 * j:16 * j + 16]
    vseed = np.zeros((1, CAUG), f32)
    vseed[0, VW * np.arange(GH) + 16] = 1.0

    # rope freq pattern [3, 64] for the COMPACT layout: row r = 16m + d of a
    # 64-row block belongs to head-in-block m -> h = m%2, pair w = d//2
    fr = rope_freqs.astype(f32)            # [3, 2, 8]
    fpat = np.zeros((3, 64), f32)
    for r in range(64):
        fpat[:, r] = fr[:, (r // 16) % 2, (r % 16) // 2]

    # spread matrix: compact rows (16/head) -> 32-aligned slots; two stacked
    # copies so odd 64-row slabs can use base partition 64
    esp2 = np.zeros((128, 128), f32)
    for k in range(64):
        m = 32 * (k // 16) + (k % 16)
        esp2[k, m] = 1.0
        esp2[64 + k, m] = 1.0

    # normalization broadcast: row j of rden -> 16 consecutive emb rows
    e24 = np.zeros((GH, C), f32)
    e24[idx // 16, idx] = 1.0

    onesrow = np.ones((1, 128), f32)
    ident = np.eye(128, dtype=f32)
    def pack(w):
        # [384, cols] -> [128, 3*cols]: row p = concat_s w[128 s + p]
        cols = w.shape[1]
        return np.ascontiguousarray(
            w.reshape(3, 128, cols).transpose(1, 0, 2).reshape(128, 3 * cols)
            .astype(f32))

    return dict(
        wq=pack(Wq), wqp=pack(Wqp), wk=pack(Wk), wkp=pack(Wkp),
        wvil=pack(Wvil), wo=pack(Wo),
        vseed=vseed, fpat=fpat, esp2=esp2, e24=e24,
        onesrow=onesrow, ident=ident,
    )


def _build_nc():
    import concourse.bacc as bacc
    import concourse.tile as tile
    import concourse.mybir as mybir
    from contextlib import ExitStack

    f32 = mybir.dt.float32
    fmm = mybir.dt.float32r if _F32R_PROJ else f32
    AF = mybir.ActivationFunctionType

    nc = bacc.Bacc("TRN2", target_bir_lowering=False)

    x_d = nc.dram_tensor("x", [128, 2 * C], f32, kind="ExternalInput")
    posT_d = nc.dram_tensor("posT", [3, NPC], f32, kind="ExternalInput")
    wq_d = nc.dram_tensor("wq", [128, 3 * C], fmm, kind="ExternalInput")
    wqp_d = nc.dram_tensor("wqp", [128, 3 * C], fmm, kind="ExternalInput")
    wk_d = nc.dram_tensor("wk", [128, 3 * C], fmm, kind="ExternalInput")
    wkp_d = nc.dram_tensor("wkp", [128, 3 * C], fmm, kind="ExternalInput")
    wvil_d = nc.dram_tensor("wvil", [128, 3 * CAUG], fmm, kind="ExternalInput")
    wo_d = nc.dram_tensor("wo", [128, 3 * C], fmm, kind="ExternalInput")
    vseed_d = nc.dram_tensor("vseed", [1, CAUG], fmm, kind="ExternalInput")
    fpat_d = nc.dram_tensor("fpat", [3, 64], f32, kind="ExternalInput")
    esp2_d = nc.dram_tensor("esp2", [128, 128], fmm, kind="ExternalInput")
    e24_d = nc.dram_tensor("e24", [GH, C], f32, kind="ExternalInput")
    ones_d = nc.dram_tensor("onesrow", [1, 128], fmm, kind="ExternalInput")
    ident_d = nc.dram_tensor("ident", [128, 128], f32, kind="ExternalInput")
    y_d = nc.dram_tensor("y", [NPC, C], f32, kind="ExternalOutput")

    ctx = ExitStack()
    with tile.TileContext(nc) as tc, ctx:
        consts = ctx.enter_context(tc.tile_pool(name="consts", bufs=1))
        wpool = ctx.enter_context(tc.tile_pool(name="weights", bufs=1))
        sb = ctx.enter_context(tc.tile_pool(name="sbuf", bufs=1))
        # general psum: shared tag -> 4 recycled 1-bank slots
        ps_gp = ctx.enter_context(tc.tile_pool(name="ps_gp", bufs=2, space="PSUM"))
        ps_att = ctx.enter_context(tc.tile_pool(name="ps_att", bufs=1, space="PSUM"))
        ps_av = ctx.enter_context(tc.tile_pool(name="ps_av", bufs=2, space="PSUM"))

        def gpt(shape):
            return ps_gp.tile(shape, f32, tag="pp", name="pp")

        # ---- inputs first (x feeds the transposes while weights stream),
        # weights split across the two HWDGE queues (sync + scalar) ----
        xsb = sb.tile([128, 2, C], f32, tag="x")
        nc.sync.dma_start(out=xsb, in_=x_d.rearrange("p (s e) -> p s e", s=2))
        ident = consts.tile([128, 128], f32, tag="ident")
        nc.scalar.dma_start(out=ident, in_=ident_d[:])
        posT = consts.tile([3, NPC], f32, tag="posT")
        nc.scalar.dma_start(out=posT, in_=posT_d[:])
        fpat = consts.tile([3, 64], f32, tag="fpat")
        nc.scalar.dma_start(out=fpat, in_=fpat_d[:])
        esp2 = consts.tile([128, 128], fmm, tag="esp2")
        nc.scalar.dma_start(out=esp2, in_=esp2_d[:])
        e24 = consts.tile([GH, C], f32, tag="e24")
        nc.scalar.dma_start(out=e24, in_=e24_d[:])
        vseed = consts.tile([1, CAUG], fmm, tag="vseed")
        nc.scalar.dma_start(out=vseed, in_=vseed_d[:])
        onesrow = consts.tile([1, 128], fmm, tag="ones")
        nc.scalar.dma_start(out=onesrow, in_=ones_d[:])

        def load_w(dram, cols, tag, eng):
            t = wpool.tile([128, 3, cols], fmm, tag=tag)
            dv = dram.rearrange("p (s e) -> p s e", s=3)
            for s in range(3):
                eng.dma_start(out=t[:, s, :], in_=dv[:, s, :])
            return t
        wq = load_w(wq_d, C, "wq", nc.sync)
        wqp = load_w(wqp_d, C, "wqp", nc.scalar)
        wk = load_w(wk_d, C, "wk", nc.sync)
        wkp = load_w(wkp_d, C, "wkp", nc.scalar)
        wvil = load_w(wvil_d, CAUG, "wvil", nc.sync)
        wo = load_w(wo_d, C, "wo", nc.scalar)

        # ---- X^T [384, 256] via PE transposes ----
        xT = []
        for j in range(3):
            t = sb.tile([128, NPC], fmm, tag=f"xT{j}")
            for i in range(2):
                pst = gpt([128, 128])
                nc.tensor.transpose(
                    out=pst, in_=xsb[:, i, 128 * j:128 * j + 128], identity=ident)
                nc.vector.tensor_copy(out=t[:, 128 * i:128 * i + 128], in_=pst)
            xT.append(t)

        # ---- theta pattern + cos/sin [64, 256] -> stacked [128, 256] ----
        thps = gpt([64, NPC])
        nc.tensor.matmul(out=thps, lhsT=fpat, rhs=posT, start=True, stop=True)
        # range-reduce into [-pi, pi] for the scalar-engine Sin table:
        #   tr = t - 2pi*rint(t/2pi)  (f32<->i32 convert rounds to nearest)
        PI = float(np.pi)
        thc = sb.tile([64, NPC], f32, tag="thc")
        nc.vector.tensor_scalar_add(thc, thps, PI / 2)   # cos(t) = sin(t + pi/2)

        def range_reduce(src, tag):
            # robust to int-convert rounding mode (trunc on sim, rint on hw):
            # u = t+16pi > 0; v = u - 2pi*cvt(u/2pi) in [-pi, 2pi); then
            # subtract 2pi where v > pi  ->  [-pi, pi]
            u = sb.tile([64, NPC], f32, tag=f"u{tag}", name="u")
            nc.vector.tensor_scalar_add(u, src, 16 * PI)
            m1 = sb.tile([64, NPC], f32, tag=f"m1{tag}", name="m1")
            nc.vector.tensor_scalar_mul(m1, u, 1.0 / (2 * PI))
            ni = sb.tile([64, NPC], mybir.dt.int32, tag=f"ni{tag}", name="ni")
            nc.vector.tensor_copy(out=ni, in_=m1)
            nf = sb.tile([64, NPC], f32, tag=f"nf{tag}", name="nf")
            nc.vector.tensor_copy(out=nf, in_=ni)
            v = sb.tile([64, NPC], f32, tag=f"v{tag}", name="v")
            nc.vector.scalar_tensor_tensor(
                out=v, in0=nf, scalar=-2 * PI, in1=u,
                op0=mybir.AluOpType.mult, op1=mybir.AluOpType.add)
            mk = sb.tile([64, NPC], f32, tag=f"mk{tag}", name="mk")
            nc.vector.tensor_scalar(out=mk, in0=v, scalar1=PI, scalar2=None,
                                    op0=mybir.AluOpType.is_gt)
            red = sb.tile([64, NPC], f32, tag=f"red{tag}", name="red")
            nc.vector.scalar_tensor_tensor(
                out=red, in0=mk, scalar=-2 * PI, in1=v,
                op0=mybir.AluOpType.mult, op1=mybir.AluOpType.add)
            return red

        thr_s = range_reduce(thps, "s")
        thr_c = range_reduce(thc, "c")
        cpat = sb.tile([64, NPC], f32, tag="cpat")
        nc.scalar.activation(out=cpat, in_=thr_c, func=AF.Sin)
        spat = sb.tile([64, NPC], f32, tag="spat")
        nc.scalar.activation(out=spat, in_=thr_s, func=AF.Sin)
        cosf = sb.tile([128, NPC], f32, tag="cosf")
        sinf = sb.tile([128, NPC], f32, tag="sinf")
        for half in range(2):
            nc.sync.dma_start(out=cosf[64 * half:64 * half + 64, :], in_=cpat)
            nc.sync.dma_start(out=sinf[64 * half:64 * half + 64, :], in_=spat)

        # ---- projections (transposed) + compact RoPE + spread, per m-slab ----
        def proj_m(w, m):
            ps = gpt([128, NPC])
            for k in range(3):
                nc.tensor.matmul(
                    out=ps,
                    lhsT=w[:, k, 128 * m:128 * m + 128],
                    rhs=xT[k],
                    start=(k == 0), stop=(k == 2))
            return ps

        def rope_spread(w, wp, tag):
            """rotated + spread [768, 256] as 6 sbuf tiles."""
            spread = []
            for m in range(3):
                qt = proj_m(w, m)
                qpt = proj_m(wp, m)
                a = sb.tile([128, NPC], f32, tag=f"ra{tag}{m}")
                b = sb.tile([128, NPC], f32, tag=f"rb{tag}{m}")
                nc.vector.tensor_mul(out=a, in0=qt, in1=cosf)
                nc.vector.tensor_mul(out=b, in0=qpt, in1=sinf)
                rot = sb.tile([128, NPC], fmm, tag=f"rot{tag}{m}")
                nc.vector.tensor_add(out=rot, in0=a, in1=b)
                for half in range(2):
                    sp = gpt([128, NPC])
                    nc.tensor.matmul(
                        out=sp,
                        lhsT=esp2[64 * half:64 * half + 64, :],
                        rhs=rot[64 * half:64 * half + 64, :],
                        start=True, stop=True)
                    t = sb.tile([128, NPC], f32, tag=f"sps{tag}{2 * m + half}")
                    nc.vector.tensor_copy(out=t, in_=sp)
                    spread.append(t)
            return spread

        qsp = rope_spread(wq, wqp, "q")
        ksp = rope_spread(wk, wkp, "k")

        # ---- V_aug [256, 408] untransposed (+ ones cols via K=1 matmul) ----
        vau = []
        for i in range(2):
            ps = gpt([128, CAUG])
            for k in range(3):
                nc.tensor.matmul(
                    out=ps,
                    lhsT=xT[k][:, 128 * i:128 * i + 128],
                    rhs=wvil[:, k, :],
                    start=(k == 0), stop=False)
            nc.tensor.matmul(
                out=ps, lhsT=onesrow, rhs=vseed,
                start=False, stop=True)
            t = sb.tile([128, CAUG], f32, tag=f"vau{i}")
            nc.vector.tensor_copy(out=t, in_=ps)
            vau.append(t)

        # ---- scores S^T + exp, per graph-pair.
        # Concurrent row-tiled matmuls MUST land in distinct PSUM banks:
        # head gh -> bank gh%4 (512-col block), col 64*(gh//4), rows 64*g01.
        def scol(gh):
            return 512 * (gh % 4) + 64 * (gh // 4)

        expst = []
        for pair in range(2):
            stp = ps_att.tile([128, 4 * 512], f32, tag="stps")
            for gh in range(GH):
                tilei, slot = divmod(gh, 4)
                lo = 32 * slot
                for g01 in range(2):
                    g = 2 * pair + g01
                    nc.tensor.matmul(
                        out=stp[64 * g01:64 * g01 + 64, scol(gh):scol(gh) + 64],
                        lhsT=ksp[tilei][lo:lo + 16, 64 * g:64 * g + 64],
                        rhs=qsp[tilei][lo:lo + 16, 64 * g:64 * g + 64],
                        start=True, stop=True,
                        tile_position=(lo, 64 * g01))
            et = sb.tile([128, 4 * 512], f32, tag=f"expst{pair}")
            for b in range(4):
                nc.scalar.activation(
                    out=et[:, 512 * b:512 * b + 384],
                    in_=stp[:, 512 * b:512 * b + 384],
                    func=AF.Exp, scale=0.25)
            expst.append(et)

        # ---- AV (+den row): per (quad, parity) [128, 128] psum tiles so the
        # two concurrent row groups (graph parities) use distinct banks;
        # head slot 32*(gh%4) rows, col 64*(g//2) ----
        avsb = sb.tile([128, 6 * 256], f32, tag="avsb")
        dens = sb.tile([GH, NPC], f32, tag="dens")
        og = [sb.tile([128, NPC], f32, tag=f"og{m}", name="og") for m in range(3)]
        for qd in range(6):
            avt = [ps_av.tile([128, 128], f32, tag="av", name="av")
                   for _ in range(2)]
            nc.vector.memset(avt[0], 0.0)
            nc.vector.memset(avt[1], 0.0)
            for a in range(4):
                gh = 4 * qd + a
                for g in range(GPC):
                    pair, g01 = divmod(g, 2)
                    lo = 64 * g01
                    nc.tensor.matmul(
                        out=avt[g01][32 * a:32 * a + VW,
                                     64 * (g // 2):64 * (g // 2) + 64],
                        lhsT=vau[pair][lo:lo + 64, VW * gh:VW * gh + VW],
                        rhs=expst[pair][lo:lo + 64, scol(gh):scol(gh) + 64],
                        start=True, stop=True,
                        tile_position=(lo, 32 * a))
            for g01 in range(2):
                nc.vector.tensor_copy(
                    out=avsb[:, 256 * qd + 128 * g01:256 * qd + 128 * g01 + 128],
                    in_=avt[g01])
            # per-quad gathers overlap the remaining AV quads; columns stay in
            # avsb's (g01, pair, i) node order (final DRAM write undoes it)
            cs = slice(256 * qd, 256 * qd + 256)
            for a in range(4):
                gh = 4 * qd + a
                dst, row = divmod(16 * gh, 128)
                nc.scalar.dma_start(
                    out=dens[gh:gh + 1, :],
                    in_=avsb[32 * a + 16:32 * a + 17, cs])
                (nc.sync if a % 2 else nc.scalar).dma_start(
                    out=og[dst][row:row + 16, :],
                    in_=avsb[32 * a:32 * a + 16, cs])
        rden = sb.tile([GH, NPC], f32, tag="rden")
        nc.vector.reciprocal(out=rden, in_=dens)
        onrm = []
        for m in range(3):
            rt = gpt([128, NPC])
            nc.tensor.matmul(
                out=rt, lhsT=e24[:, 128 * m:128 * m + 128],
                rhs=rden, start=True, stop=True)
            t = sb.tile([128, NPC], fmm, tag=f"onrm{m}")
            nc.vector.tensor_mul(out=t, in0=og[m], in1=rt)
            onrm.append(t)

        # ---- y = O_norm @ Wo ----
        for i in range(2):
            yps = gpt([128, C])
            for k in range(3):
                nc.tensor.matmul(
                    out=yps,
                    lhsT=onrm[k][:, 128 * i:128 * i + 128],
                    rhs=wo[:, k, :],
                    start=(k == 0), stop=(k == 2))
            ysb = sb.tile([128, C], f32, tag=f"ysb{i}", name="ysb")
            nc.vector.tensor_copy(out=ysb, in_=yps)
            # node column order downstream of avsb is (g01, pair, i); psum
            # y-tile i covers g01 == i, rows (pair, i64) -> node 128*pair + 64*i + i64
            nc.sync.dma_start(
                out=y_d.rearrange("(pr b i) e -> b pr i e", pr=2, b=2, i=64)[i],
                in_=ysb)

    nc.compile()
    return nc


def _get_nc():
    if "nc" not in _CACHE:
        _CACHE["nc"] = _build_nc()
    return _CACHE["nc"]


def make_in_maps(inputs):
    x = np.asarray(inputs["x"], np.float32)
    pos = np.asarray(inputs["pos"], np.float32)
    prep = _host_prep(np.asarray(inputs["Wq"], np.float32),
                      np.asarray(inputs["Wk"], np.float32),
                      np.asarray(inputs["Wv"], np.float32),
                      np.asarray(inputs["Wo"], np.float32),
                      np.asarray(inputs["rope_freqs"], np.float32))
    in_maps = []
    for c in range(NCORES):
        sl = slice(c * NPC, (c + 1) * NPC)
        m = dict(prep)
        xs = x[sl]
        m["x"] = np.ascontiguousarray(
            xs.reshape(2, 128, C).transpose(1, 0, 2).reshape(128, 2 * C))
        m["posT"] = np.ascontiguousarray(pos[sl].T)
        in_maps.append(m)
    return in_maps


def kernel(**inputs):
    from concourse.bass_utils import run_bass_kernel_spmd

    in_maps = make_in_maps(inputs)

    nc = _get_nc()
    res = run_bass_kernel_spmd(nc, in_maps, core_ids=list(range(NCORES)))
    out = np.concatenate([res.results[c]["y"] for c in range(NCORES)], axis=0)
    return out.astype(np.float32)



# revision 26
# speedup vs baseline: 1.2623x; 1.2623x over previous
"""PlatonicConv (graph-mode attention) Trainium2 Bass kernel.

Math (per graph of 64 fully-connected nodes, 24 group-heads of dim 16):
  q/k/v = x @ W; RoPE(q, k) from pos; S = q.k^T/4; softmax over dst;
  out = A @ v; y = out @ Wo.  32 graphs -> data-parallel over 8 cores.

Layout choices (per core: 4 graphs, 256 nodes):
  * Attention side lives transposed ([feature, node]); x is transposed on host.
  * RoPE cos/sin caches are host-precomputed patterns [128, 256]; the rotation
    partner-swap + sign is folded into a second spread matrix (esp2b), so the
    pair-swapped projections (Wqp/Wkp) are not needed: only one projection per
    q/k slab, then  spread = esp2a^T (qt*cos) + esp2b^T (qt*sin)  accumulated
    in PSUM (this also absorbs the rope add).
  * Heads are "spread" to 32-aligned partition slots so score matmuls pack
    4-way into the PE array via tile_position row groups.
  * Softmax is max-free (scores are O(1) by construction); the denominator
    comes for free as a 17th row of each AV matmul via an interleaved
    ones-column in the V weights.
  * The spread->compact "unspread" after AV is a single selection matmul per
    head-quad (rows 0-3 = denominators, 4-67 = compact out); og moves by one
    DMA per quad and the denominators are reciprocal'd straight out of PSUM
    by the scalar engine.
"""

import numpy as np

G = 12
H = 2
D = 16
GH = 24          # G * H group-heads
C = 384          # in/emb/out channels
NG = 32          # graphs
NPG = 64         # nodes per graph
N = NG * NPG
NCORES = 8
GPC = NG // NCORES   # graphs per core = 4
NPC = GPC * NPG      # nodes per core = 256
VW = 17              # V block width (16 + ones col)
CAUG = GH * VW       # 408
CSTW = 1116          # packed consts width

_CACHE = {}


def _host_prep(Wq, Wk, Wv, Wo, rope_freqs):
    f32 = np.float32

    # esp2a: compact row (within 64-block) -> 32-aligned spread slot; stacked
    # twice so odd 64-row slabs can use base partition 64
    esp2a = np.zeros((128, 128), f32)
    for k in range(64):
        m = 32 * (k // 16) + (k % 16)
        esp2a[k, m] = 1.0
        esp2a[64 + k, m] = 1.0
    # esp2b = P^T esp2a: P = rope pair-swap (d even<->odd) with sign(-1 on even)
    p64 = np.arange(64) ^ 1
    s64 = np.where(np.arange(64) % 2 == 0, -1.0, 1.0).astype(f32)
    esp2b = np.zeros((128, 128), f32)
    for c in range(64):
        r = p64[c]
        esp2b[c, :] = s64[r] * esp2a[r, :]
        esp2b[64 + c, :] = s64[r] * esp2a[64 + r, :]

    # unspread selection, parity-matched so every downstream engine move is
    # partition-shift-free:
    #   even quads: og -> out rows 0-63, den -> rows 64-67
    #   odd quads:  og -> out rows 64-127, den -> rows 0-3
    selE = np.zeros((128, 68), f32)
    selO = np.zeros((128, 128), f32)
    for a in range(4):
        selE[32 * a + 16, 64 + a] = 1.0
        selO[32 * a + 16, a] = 1.0
        for d in range(16):
            selE[32 * a + d, 16 * a + d] = 1.0
            selO[32 * a + d, 64 + 16 * a + d] = 1.0

    # e68: rden2 rows (64-67 = even-quad dens -> out rows 0-63; 0-3 = odd-quad
    # dens -> out rows 64-127), broadcast to 16 consecutive emb rows each
    e68 = np.zeros((68, 128), f32)
    for i in range(64):
        e68[64 + i // 16, i] = 1.0
        e68[i // 16, 64 + i] = 1.0

    # packed consts [128, 1052]
    cst = np.zeros((128, CSTW), f32)
    cst[:, 0:128] = esp2a
    cst[:, 128:256] = esp2b
    cst[:, 256:324] = selE
    cst[:, 324:452] = selO
    cst[0:68, 452:580] = e68
    cst[0, 580:708] = 1.0                        # onesrow
    cst[0, 708 + VW * np.arange(GH) + 16] = 1.0  # vseed (cols 708:1116)

    # V interleaved with a ones column per head: block j = [Wv head j | 0]
    Wvil = np.zeros((C, CAUG), f32)
    for j in range(GH):
        Wvil[:, VW * j:VW * j + 16] = Wv[:, 16 * j:16 * j + 16]

    def pack(w):
        # [384, cols] -> [128, 3*cols]: col block s = w[128 s : 128 s + 128]
        cols = w.shape[1]
        return np.ascontiguousarray(
            w.reshape(3, 128, cols).transpose(1, 0, 2).reshape(128, 3 * cols)
            .astype(f32))

    return dict(
        wq=pack(Wq), wk=pack(Wk), wvil=pack(Wvil), wo=pack(Wo), cst=cst,
        e68=np.ascontiguousarray(e68),
    )


def _rope_cache(pos, rope_freqs):
    # cos/sin patterns [128, 256]: row r (mod 64) = 16 m + d -> head h = m%2,
    # freq index d//2; two stacked 64-row copies
    f32 = np.float32
    theta = np.einsum('ns,shf->nhf', pos.astype(f32), rope_freqs.astype(f32))
    r = np.arange(64)
    h = (r // 16) % 2
    f = (r % 16) // 2
    cpat = np.cos(theta[:, h, f]).T.astype(f32)   # [64, 256]
    spat = np.sin(theta[:, h, f]).T.astype(f32)
    cs = np.empty((128, 2 * NPC), f32)
    cs[0:64, 0:NPC] = cpat
    cs[64:128, 0:NPC] = cpat
    cs[0:64, NPC:] = spat
    cs[64:128, NPC:] = spat
    return cs


def _build_nc():
    import concourse.bacc as bacc
    import concourse.tile as tile
    import concourse.mybir as mybir
    from contextlib import ExitStack

    f32 = mybir.dt.float32
    fmm = mybir.dt.float32r
    AF = mybir.ActivationFunctionType

    nc = bacc.Bacc("TRN2", target_bir_lowering=False)

    xT_d = nc.dram_tensor("xT", [128, 3 * NPC], fmm, kind="ExternalInput")
    cs_d = nc.dram_tensor("cs", [128, 2 * NPC], f32, kind="ExternalInput")
    wq_d = nc.dram_tensor("wq", [128, 3 * C], fmm, kind="ExternalInput")
    wk_d = nc.dram_tensor("wk", [128, 3 * C], fmm, kind="ExternalInput")
    wvil_d = nc.dram_tensor("wvil", [128, 3 * CAUG], fmm, kind="ExternalInput")
    wo_d = nc.dram_tensor("wo", [128, 3 * C], fmm, kind="ExternalInput")
    cst_d = nc.dram_tensor("cst", [128, CSTW], fmm, kind="ExternalInput")
    e68_d = nc.dram_tensor("e68", [68, 128], f32, kind="ExternalInput")
    y_d = nc.dram_tensor("y", [NPC, C], f32, kind="ExternalOutput")

    ctx = ExitStack()
    with tile.TileContext(nc) as tc, ctx:
        consts = ctx.enter_context(tc.tile_pool(name="consts", bufs=1))
        sb = ctx.enter_context(tc.tile_pool(name="sbuf", bufs=1))
        # general psum: shared tag -> recycled 1-bank slots
        ps_gp = ctx.enter_context(tc.tile_pool(name="ps_gp", bufs=2, space="PSUM"))
        ps_att = ctx.enter_context(tc.tile_pool(name="ps_att", bufs=1, space="PSUM"))
        ps_av = ctx.enter_context(tc.tile_pool(name="ps_av", bufs=2, space="PSUM"))

        def gpt(shape):
            return ps_gp.tile(shape, f32, tag="pp", name="pp")

        # ---- input DMAs: one descriptor each, split across the two queues ----
        xT = consts.tile([128, 3, NPC], fmm, tag="xT")
        nc.sync.dma_start(out=xT, in_=xT_d.rearrange("p (s e) -> p s e", s=3))
        cs = consts.tile([128, 2, NPC], f32, tag="cs")
        nc.scalar.dma_start(out=cs, in_=cs_d.rearrange("p (s e) -> p s e", s=2))
        cst = consts.tile([128, CSTW], fmm, tag="cst")
        nc.scalar.dma_start(out=cst, in_=cst_d[:])

        def load_w(dram, cols, tag, eng):
            t = consts.tile([128, 3, cols], fmm, tag=tag)
            eng.dma_start(out=t, in_=dram.rearrange("p (s e) -> p s e", s=3))
            return t
        wq = load_w(wq_d, C, "wq", nc.sync)
        wk = load_w(wk_d, C, "wk", nc.scalar)
        wvil = load_w(wvil_d, CAUG, "wvil", nc.sync)
        wo = load_w(wo_d, C, "wo", nc.scalar)
        e68 = consts.tile([68, 128], f32, tag="e68")
        nc.scalar.dma_start(out=e68, in_=e68_d[:])

        cosf = cs[:, 0, :]
        sinf = cs[:, 1, :]
        esp2a = cst[:, 0:128]
        esp2b = cst[:, 128:256]
        selE = cst[:, 256:324]
        selO = cst[:, 324:452]
        onesrow = cst[0:1, 580:708]
        vseed = cst[0:1, 708:708 + CAUG]

        # ---- projections (transposed) + RoPE + spread, per 128-row m-slab.
        # spread = esp2a^T (qt*cos) + esp2b^T (qt*sin), accumulated in psum.
        def rope_spread(w, tag):
            spread = []
            for m in range(3):
                ps = gpt([128, NPC])
                for k in range(3):
                    nc.tensor.matmul(
                        out=ps,
                        lhsT=w[:, k, 128 * m:128 * m + 128],
                        rhs=xT[:, k, :],
                        start=(k == 0), stop=(k == 2))
                a = sb.tile([128, NPC], fmm, tag=f"ra{tag}{m}")
                b = sb.tile([128, NPC], fmm, tag=f"rb{tag}{m}")
                nc.vector.tensor_mul(out=a, in0=ps, in1=cosf)
                nc.vector.tensor_mul(out=b, in0=ps, in1=sinf)
                for half in range(2):
                    hs = slice(64 * half, 64 * half + 64)
                    sp = gpt([128, NPC])
                    nc.tensor.matmul(out=sp, lhsT=esp2a[hs, :], rhs=a[hs, :],
                                     start=True, stop=False)
                    nc.tensor.matmul(out=sp, lhsT=esp2b[hs, :], rhs=b[hs, :],
                                     start=False, stop=True)
                    # f32 (not f32r): score matmuls use tile_position dst
                    # offsets that are invalid for f32r operands
                    t = sb.tile([128, NPC], f32, tag=f"sps{tag}{2 * m + half}")
                    nc.vector.tensor_copy(out=t, in_=sp)
                    spread.append(t)
            return spread

        qsp = rope_spread(wq, "q")
        ksp = rope_spread(wk, "k")

        # ---- V_aug [256, 408] untransposed (+ ones cols via K=1 matmul) ----
        vau = []
        for i in range(2):
            ps = gpt([128, CAUG])
            for k in range(3):
                nc.tensor.matmul(
                    out=ps,
                    lhsT=xT[:, k, 128 * i:128 * i + 128],
                    rhs=wvil[:, k, :],
                    start=(k == 0), stop=False)
            nc.tensor.matmul(
                out=ps, lhsT=onesrow, rhs=vseed,
                start=False, stop=True)
            t = sb.tile([128, CAUG], f32, tag=f"vau{i}")
            nc.vector.tensor_copy(out=t, in_=ps)
            vau.append(t)

        # ---- scores S^T + exp, per graph-pair.
        # Concurrent row-tiled matmuls MUST land in distinct PSUM banks:
        # head gh -> bank gh%4 (512-col block), col 64*(gh//4), rows 64*g01.
        def scol(gh):
            return 512 * (gh % 4) + 64 * (gh // 4)

        expst = []
        for pair in range(2):
            stp = ps_att.tile([128, 4 * 512], f32, tag="stps")
            for gh in range(GH):
                tilei, slot = divmod(gh, 4)
                lo = 32 * slot
                for g01 in range(2):
                    g = 2 * pair + g01
                    nc.tensor.matmul(
                        out=stp[64 * g01:64 * g01 + 64, scol(gh):scol(gh) + 64],
                        lhsT=ksp[tilei][lo:lo + 16, 64 * g:64 * g + 64],
                        rhs=qsp[tilei][lo:lo + 16, 64 * g:64 * g + 64],
                        start=True, stop=True,
                        tile_position=(lo, 64 * g01))
            et = sb.tile([128, 4 * 512], f32, tag=f"expst{pair}")
            for b in range(4):
                nc.scalar.activation(
                    out=et[:, 512 * b:512 * b + 384],
                    in_=stp[:, 512 * b:512 * b + 384],
                    func=AF.Exp, scale=0.25)
            expst.append(et)

        # ---- AV (+den row): per (quad, parity) [128, 128] psum tiles so the
        # two concurrent row groups (graph parities) use distinct banks;
        # head slot 32*(gh%4) rows, col 64*(g//2).
        # Then unspread via one sel matmul: rows 0-3 = den, 4-67 = compact out.
        avsb = sb.tile([128, 6 * 256], fmm, tag="avsb")
        og = [sb.tile([128, NPC], fmm, tag=f"og{m}", name="og") for m in range(3)]
        # rden2: quad pair p -> cols 256p; even quad dens at rows 64-67,
        # odd at rows 0-3 (parity-matched to the recip source partitions).
        # Rows 4-63 are never written but read by the e68 matmul: zero once.
        rden2 = sb.tile([68, 3 * NPC], f32, tag="rden")
        nc.vector.memset(rden2, 0.0)
        for qd in range(6):
            avt = [ps_av.tile([128, 128], f32, tag="av", name="av")
                   for _ in range(2)]
            if qd < 2:  # slots are recycled afterwards (stale-but-finite rows)
                nc.vector.memset(avt[0], 0.0)
                nc.vector.memset(avt[1], 0.0)
            for g in range(GPC):
                pair, g01 = divmod(g, 2)
                lo = 64 * g01
                for a in range(4):
                    gh = 4 * qd + a
                    nc.tensor.matmul(
                        out=avt[g01][32 * a:32 * a + VW,
                                     64 * (g // 2):64 * (g // 2) + 64],
                        lhsT=vau[pair][lo:lo + 64, VW * gh:VW * gh + VW],
                        rhs=expst[pair][lo:lo + 64, scol(gh):scol(gh) + 64],
                        start=True, stop=True,
                        tile_position=(lo, 32 * a))
            cq = slice(256 * qd, 256 * qd + 256)
            for g01 in range(2):
                nc.vector.tensor_copy(
                    out=avsb[:, 256 * qd + 128 * g01:256 * qd + 128 * g01 + 128],
                    in_=avt[g01])
            odd = qd % 2
            ups = gpt([128, NPC])
            nc.tensor.matmul(out=ups[0:68, :] if not odd else ups,
                             lhsT=selO if odd else selE, rhs=avsb[:, cq],
                             start=True, stop=True)
            ohs = slice(64 * odd, 64 * odd + 64)
            dhs = slice(64 - 64 * odd, 68 - 64 * odd)
            rp = slice(NPC * (qd // 2), NPC * (qd // 2) + NPC)
            nc.vector.reciprocal(out=rden2[dhs, rp], in_=ups[dhs, :])
            # node columns stay in (g01, pair, i) order (final DRAM write undoes)
            nc.scalar.activation(out=og[qd // 2][ohs, :], in_=ups[ohs, :],
                                 func=AF.Copy)

        # ---- normalize + y = O_norm @ Wo ----
        onrm = []
        for m in range(3):
            rt = gpt([128, NPC])
            rp = slice(NPC * m, NPC * m + NPC)
            nc.tensor.matmul(out=rt, lhsT=e68, rhs=rden2[:, rp],
                             start=True, stop=True)
            t = sb.tile([128, NPC], fmm, tag=f"onrm{m}")
            nc.vector.tensor_mul(out=t, in0=og[m], in1=rt)
            onrm.append(t)

        for i in range(2):
            yps = gpt([128, C])
            for m in range(3):
                nc.tensor.matmul(
                    out=yps,
                    lhsT=onrm[m][:, 128 * i:128 * i + 128],
                    rhs=wo[:, m, :],
                    start=(m == 0), stop=(m == 2))
            ysb = sb.tile([128, C], f32, tag=f"ysb{i}", name="ysb")
            nc.vector.tensor_copy(out=ysb, in_=yps)
            # node column order downstream of avsb is (g01, pair, i); psum
            # y-tile i covers g01 == i, rows (pair, i64) -> node 128*pair + 64*i + i64
            nc.sync.dma_start(
                out=y_d.rearrange("(pr b i) e -> b pr i e", pr=2, b=2, i=64)[i],
                in_=ysb)

    nc.compile()
    return nc


def _get_nc():
    if "nc" not in _CACHE:
        _CACHE["nc"] = _build_nc()
    return _CACHE["nc"]


def make_in_maps(inputs):
    x = np.asarray(inputs["x"], np.float32)
    pos = np.asarray(inputs["pos"], np.float32)
    freqs = np.asarray(inputs["rope_freqs"], np.float32)
    prep = _host_prep(np.asarray(inputs["Wq"], np.float32),
                      np.asarray(inputs["Wk"], np.float32),
                      np.asarray(inputs["Wv"], np.float32),
                      np.asarray(inputs["Wo"], np.float32),
                      freqs)
    in_maps = []
    for c in range(NCORES):
        sl = slice(c * NPC, (c + 1) * NPC)
        m = dict(prep)
        xs = np.ascontiguousarray(x[sl].T)          # [384, 256]
        m["xT"] = np.ascontiguousarray(
            xs.reshape(3, 128, NPC).transpose(1, 0, 2).reshape(128, 3 * NPC))
        m["cs"] = _rope_cache(pos[sl], freqs)
        in_maps.append(m)
    return in_maps


def kernel(**inputs):
    from concourse.bass_utils import run_bass_kernel_spmd

    in_maps = make_in_maps(inputs)

    nc = _get_nc()
    res = run_bass_kernel_spmd(nc, in_maps, core_ids=list(range(NCORES)))
    out = np.concatenate([res.results[c]["y"] for c in range(NCORES)], axis=0)
    return out.astype(np.float32)


# revision 38
# speedup vs baseline: 1.7236x; 1.3655x over previous
"""PlatonicConv (graph-mode attention) Trainium2 Bass kernel.

Math (per graph of 64 fully-connected nodes, 24 group-heads of dim 16):
  q/k/v = x @ W; RoPE(q, k) from pos; S = q.k^T/4; softmax over dst;
  out = A @ v; y = out @ Wo.  32 graphs -> data-parallel over 8 cores.

Layout choices (per core: 4 graphs, 256 nodes):
  * Attention side lives transposed ([feature, node]); x is transposed on host.
  * RoPE cos/sin caches are host-precomputed patterns [128, 256]; the rotation
    partner-swap + sign is folded into a second spread matrix (esp2b), so the
    pair-swapped projections (Wqp/Wkp) are not needed: only one projection per
    q/k slab, then  spread = esp2a^T (qt*cos) + esp2b^T (qt*sin)  accumulated
    in PSUM (this also absorbs the rope add).
  * Heads are "spread" to 32-aligned partition slots so score matmuls pack
    4-way into the PE array via tile_position row groups.
  * Softmax is max-free (scores are O(1) by construction); the denominator
    comes for free as a 17th row of each AV matmul via an interleaved
    ones-column in the V weights.
  * The spread->compact "unspread" after AV is a single selection matmul per
    head-quad (rows 0-3 = denominators, 4-67 = compact out); og moves by one
    DMA per quad and the denominators are reciprocal'd straight out of PSUM
    by the scalar engine.
"""

import numpy as np

G = 12
H = 2
D = 16
GH = 24          # G * H group-heads
C = 384          # in/emb/out channels
NG = 32          # graphs
NPG = 64         # nodes per graph
N = NG * NPG
NCORES = 8
GPC = NG // NCORES   # graphs per core = 4
NPC = GPC * NPG      # nodes per core = 256
VW = 17              # V block width (16 + ones col)
CAUG = GH * VW       # 408
CSTW = 1116          # packed consts width

_CACHE = {}


def _host_prep(Wq, Wk, Wv, Wo, rope_freqs):
    f32 = np.float32

    # esp2a: compact row (within 64-block) -> 32-aligned spread slot; stacked
    # twice so odd 64-row slabs can use base partition 64
    esp2a = np.zeros((128, 128), f32)
    for k in range(64):
        m = 32 * (k // 16) + (k % 16)
        esp2a[k, m] = 1.0
        esp2a[64 + k, m] = 1.0
    # esp2b = P^T esp2a: P = rope pair-swap (d even<->odd) with sign(-1 on even)
    p64 = np.arange(64) ^ 1
    s64 = np.where(np.arange(64) % 2 == 0, -1.0, 1.0).astype(f32)
    esp2b = np.zeros((128, 128), f32)
    for c in range(64):
        r = p64[c]
        esp2b[c, :] = s64[r] * esp2a[r, :]
        esp2b[64 + c, :] = s64[r] * esp2a[64 + r, :]

    # unspread selection, parity-matched so every downstream engine move is
    # partition-shift-free:
    #   even quads: og -> out rows 0-63, den -> rows 64-67
    #   odd quads:  og -> out rows 64-127, den -> rows 0-3
    selE = np.zeros((128, 68), f32)
    selO = np.zeros((128, 128), f32)
    for a in range(4):
        selE[32 * a + 16, 64 + a] = 1.0
        selO[32 * a + 16, a] = 1.0
        for d in range(16):
            selE[32 * a + d, 16 * a + d] = 1.0
            selO[32 * a + d, 64 + 16 * a + d] = 1.0

    # e68: rden2 rows (64-67 = even-quad dens -> out rows 0-63; 0-3 = odd-quad
    # dens -> out rows 64-127), broadcast to 16 consecutive emb rows each
    e68 = np.zeros((68, 128), f32)
    for i in range(64):
        e68[64 + i // 16, i] = 1.0
        e68[i // 16, 64 + i] = 1.0

    # packed consts [128, 1052]
    cst = np.zeros((128, CSTW), f32)
    cst[:, 0:128] = esp2a
    cst[:, 128:256] = esp2b
    cst[:, 256:324] = selE
    cst[:, 324:452] = selO
    cst[0:68, 452:580] = e68
    cst[0, 580:708] = 1.0                        # onesrow
    cst[0, 708 + VW * np.arange(GH) + 16] = 1.0  # vseed (cols 708:1116)

    # V interleaved with a ones column per head: block j = [Wv head j | 0]
    Wvil = np.zeros((C, CAUG), f32)
    for j in range(GH):
        Wvil[:, VW * j:VW * j + 16] = Wv[:, 16 * j:16 * j + 16]

    import ml_dtypes
    bf16 = ml_dtypes.bfloat16

    def pack(w):
        # [384, cols] -> [128, 3*cols]: col block s = w[128 s : 128 s + 128]
        cols = w.shape[1]
        return np.ascontiguousarray(
            w.reshape(3, 128, cols).transpose(1, 0, 2).reshape(128, 3 * cols)
            .astype(bf16))

    return dict(
        wq=pack(Wq), wk=pack(Wk), wvil=pack(Wvil), wo=pack(Wo), cst=cst,
        e68=np.ascontiguousarray(e68),
    )


def _rope_cache(pos, rope_freqs):
    # cos/sin patterns [128, 256]: row r (mod 64) = 16 m + d -> head h = m%2,
    # freq index d//2; two stacked 64-row copies
    f32 = np.float32
    theta = np.einsum('ns,shf->nhf', pos.astype(f32), rope_freqs.astype(f32))
    r = np.arange(64)
    h = (r // 16) % 2
    f = (r % 16) // 2
    cpat = np.cos(theta[:, h, f]).T.astype(f32)   # [64, 256]
    spat = np.sin(theta[:, h, f]).T.astype(f32)
    cs = np.empty((128, 2 * NPC), f32)
    cs[0:64, 0:NPC] = cpat
    cs[64:128, 0:NPC] = cpat
    cs[0:64, NPC:] = spat
    cs[64:128, NPC:] = spat
    return cs


def _build_nc():
    import concourse.bacc as bacc
    import concourse.tile as tile
    import concourse.mybir as mybir
    from contextlib import ExitStack

    f32 = mybir.dt.float32
    fmm = mybir.dt.float32r
    fb = mybir.dt.bfloat16
    AF = mybir.ActivationFunctionType

    nc = bacc.Bacc("TRN2", target_bir_lowering=False)

    xT_d = nc.dram_tensor("xT", [128, 3 * NPC], fb, kind="ExternalInput")
    cs_d = nc.dram_tensor("cs", [128, 2 * NPC], f32, kind="ExternalInput")
    wq_d = nc.dram_tensor("wq", [128, 3 * C], fb, kind="ExternalInput")
    wk_d = nc.dram_tensor("wk", [128, 3 * C], fb, kind="ExternalInput")
    wvil_d = nc.dram_tensor("wvil", [128, 3 * CAUG], fb, kind="ExternalInput")
    wo_d = nc.dram_tensor("wo", [128, 3 * C], fb, kind="ExternalInput")
    cst_d = nc.dram_tensor("cst", [128, CSTW], fmm, kind="ExternalInput")
    e68_d = nc.dram_tensor("e68", [68, 128], f32, kind="ExternalInput")
    y_d = nc.dram_tensor("y", [NPC, C], f32, kind="ExternalOutput")

    ctx = ExitStack()
    with tile.TileContext(nc) as tc, ctx:
        consts = ctx.enter_context(tc.tile_pool(name="consts", bufs=1))
        sb = ctx.enter_context(tc.tile_pool(name="sbuf", bufs=1))
        # general psum: shared tag -> recycled 1-bank slots
        ps_gp = ctx.enter_context(tc.tile_pool(name="ps_gp", bufs=2, space="PSUM"))
        ps_att = ctx.enter_context(tc.tile_pool(name="ps_att", bufs=1, space="PSUM"))
        ps_av = ctx.enter_context(tc.tile_pool(name="ps_av", bufs=2, space="PSUM"))

        def gpt(shape):
            return ps_gp.tile(shape, f32, tag="pp", name="pp")

        # ---- input DMAs, split across the two queues; weights per-slab so
        # the first projection only waits on xT + one slab ----
        xT = consts.tile([128, 3, NPC], fb, tag="xT")
        nc.sync.dma_start(out=xT, in_=xT_d.rearrange("p (s e) -> p s e", s=3))
        cs = consts.tile([128, 2, NPC], f32, tag="cs")
        nc.scalar.dma_start(out=cs, in_=cs_d.rearrange("p (s e) -> p s e", s=2))

        def load_w(dram, cols, tag, eng):
            t = consts.tile([128, 3, cols], fb, tag=tag)
            dv = dram.rearrange("p (s e) -> p s e", s=3)
            for s in range(3):
                eng.dma_start(out=t[:, s, :], in_=dv[:, s, :])
            return t
        wq = load_w(wq_d, C, "wq", nc.sync)
        cst = consts.tile([128, CSTW], fmm, tag="cst")
        nc.scalar.dma_start(out=cst, in_=cst_d[:])
        wk = load_w(wk_d, C, "wk", nc.scalar)
        wvil = load_w(wvil_d, CAUG, "wvil", nc.sync)
        wo = load_w(wo_d, C, "wo", nc.scalar)
        e68 = consts.tile([68, 128], f32, tag="e68")
        nc.scalar.dma_start(out=e68, in_=e68_d[:])

        cosf = cs[:, 0, :]
        sinf = cs[:, 1, :]
        esp2a = cst[:, 0:128]
        esp2b = cst[:, 128:256]
        selE = cst[:, 256:324]
        selO = cst[:, 324:452]
        onesrow = cst[0:1, 580:708]
        vseed = cst[0:1, 708:708 + CAUG]

        # ---- projections (transposed) + RoPE + spread, per 128-row m-slab.
        # spread = esp2a^T (qt*cos) + esp2b^T (qt*sin), accumulated in psum.
        def rope_spread(w, tag):
            spread = []
            for m in range(3):
                ps = gpt([128, NPC])
                for k in range(3):
                    nc.tensor.matmul(
                        out=ps,
                        lhsT=w[:, k, 128 * m:128 * m + 128],
                        rhs=xT[:, k, :],
                        start=(k == 0), stop=(k == 2))
                a = sb.tile([128, NPC], fmm, tag=f"ra{tag}{m}")
                b = sb.tile([128, NPC], fmm, tag=f"rb{tag}{m}")
                nc.vector.tensor_mul(out=a, in0=ps, in1=cosf)
                nc.vector.tensor_mul(out=b, in0=ps, in1=sinf)
                for half in range(2):
                    hs = slice(64 * half, 64 * half + 64)
                    sp = gpt([128, NPC])
                    nc.tensor.matmul(out=sp, lhsT=esp2a[hs, :], rhs=a[hs, :],
                                     start=True, stop=False)
                    nc.tensor.matmul(out=sp, lhsT=esp2b[hs, :], rhs=b[hs, :],
                                     start=False, stop=True)
                    # bf16 (not f32r): score matmuls use tile_position dst
                    # offsets that are invalid for f32r operands
                    t = sb.tile([128, NPC], fb, tag=f"sps{tag}{2 * m + half}")
                    if tag == "q":   # balance psum->sbuf copies across engines
                        nc.vector.tensor_copy(out=t, in_=sp)
                    else:
                        nc.scalar.activation(out=t, in_=sp, func=AF.Copy)
                    spread.append(t)
            return spread

        qsp = rope_spread(wq, "q")
        ksp = rope_spread(wk, "k")

        # ---- V_aug [256, 408] untransposed (+ ones cols via K=1 matmul) ----
        vau = []
        for i in range(2):
            ps = gpt([128, CAUG])
            for k in range(3):
                nc.tensor.matmul(
                    out=ps,
                    lhsT=xT[:, k, 128 * i:128 * i + 128],
                    rhs=wvil[:, k, :],
                    start=(k == 0), stop=False)
            nc.tensor.matmul(
                out=ps, lhsT=onesrow, rhs=vseed,
                start=False, stop=True)
            t = sb.tile([128, CAUG], fb, tag=f"vau{i}")
            nc.vector.tensor_copy(out=t, in_=ps)
            vau.append(t)

        # ---- scores S^T + exp, per graph-pair.
        # Concurrent row-tiled matmuls MUST land in distinct PSUM banks:
        # head gh -> bank gh%4 (512-col block), col 64*(gh//4), rows 64*g01.
        def scol(gh):
            return 512 * (gh % 4) + 64 * (gh // 4)

        expst = []
        for pair in range(2):
            stp = ps_att.tile([128, 4 * 512], f32, tag="stps")
            for gh in range(GH):
                tilei, slot = divmod(gh, 4)
                lo = 32 * slot
                for g01 in range(2):
                    g = 2 * pair + g01
                    nc.tensor.matmul(
                        out=stp[64 * g01:64 * g01 + 64, scol(gh):scol(gh) + 64],
                        lhsT=ksp[tilei][lo:lo + 16, 64 * g:64 * g + 64],
                        rhs=qsp[tilei][lo:lo + 16, 64 * g:64 * g + 64],
                        start=True, stop=True,
                        tile_position=(lo, 64 * g01))
            et = sb.tile([128, 4 * 512], fb, tag=f"expst{pair}")
            for b in range(4):
                nc.scalar.activation(
                    out=et[:, 512 * b:512 * b + 384],
                    in_=stp[:, 512 * b:512 * b + 384],
                    func=AF.Exp, scale=0.25)
            expst.append(et)

        # ---- AV (+den row): per (quad, parity) [128, 128] psum tiles so the
        # two concurrent row groups (graph parities) use distinct banks;
        # head slot 32*(gh%4) rows, col 64*(g//2).
        # Then unspread via one sel matmul: rows 0-3 = den, 4-67 = compact out.
        avsb = sb.tile([128, 6 * 256], fmm, tag="avsb")
        og = [sb.tile([128, NPC], fmm, tag=f"og{m}", name="og") for m in range(3)]
        # lden: ln(den); quad pair p -> cols 256p; even quad dens at rows
        # 64-67, odd at rows 0-3 (parity-matched to the Ln source partitions).
        # Rows 4-63 are never written but read by the e68 matmul: zero once.
        # (A DVE reciprocal costs ~1.7us flat, so normalization goes through
        # scalar Ln -> e68 broadcast matmul -> scalar Exp(scale=-1) instead.)
        lden = sb.tile([68, 3 * NPC], f32, tag="lden")
        nc.vector.memset(lden, 0.0)
        for qd in range(6):
            avt = [ps_av.tile([128, 128], f32, tag="av", name="av")
                   for _ in range(2)]
            if qd < 2:  # slots are recycled afterwards (stale-but-finite rows)
                nc.vector.memset(avt[0], 0.0)
                nc.vector.memset(avt[1], 0.0)
            for g in range(GPC):
                pair, g01 = divmod(g, 2)
                lo = 64 * g01
                for a in range(4):
                    gh = 4 * qd + a
                    nc.tensor.matmul(
                        out=avt[g01][32 * a:32 * a + VW,
                                     64 * (g // 2):64 * (g // 2) + 64],
                        lhsT=vau[pair][lo:lo + 64, VW * gh:VW * gh + VW],
                        rhs=expst[pair][lo:lo + 64, scol(gh):scol(gh) + 64],
                        start=True, stop=True,
                        tile_position=(lo, 32 * a))
            cq = slice(256 * qd, 256 * qd + 256)
            for g01 in range(2):
                nc.vector.tensor_copy(
                    out=avsb[:, 256 * qd + 128 * g01:256 * qd + 128 * g01 + 128],
                    in_=avt[g01])
            odd = qd % 2
            ups = gpt([128, NPC])
            nc.tensor.matmul(out=ups[0:68, :] if not odd else ups,
                             lhsT=selO if odd else selE, rhs=avsb[:, cq],
                             start=True, stop=True)
            ohs = slice(64 * odd, 64 * odd + 64)
            dhs = slice(64 - 64 * odd, 68 - 64 * odd)
            rp = slice(NPC * (qd // 2), NPC * (qd // 2) + NPC)
            nc.scalar.activation(out=lden[dhs, rp], in_=ups[dhs, :],
                                 func=AF.Ln)
            # node columns stay in (g01, pair, i) order (host gather undoes)
            nc.vector.tensor_copy(out=og[qd // 2][ohs, :], in_=ups[ohs, :])

        # ---- normalize + y = O_norm @ Wo ----
        onrm = []
        for m in range(3):
            rt = gpt([128, NPC])
            rp = slice(NPC * m, NPC * m + NPC)
            nc.tensor.matmul(out=rt, lhsT=e68, rhs=lden[:, rp],
                             start=True, stop=True)
            ert = sb.tile([128, NPC], f32, tag=f"ert{m}")
            nc.scalar.activation(out=ert, in_=rt, func=AF.Exp, scale=-1.0)
            t = sb.tile([128, NPC], fb, tag=f"onrm{m}")
            nc.vector.tensor_mul(out=t, in0=og[m], in1=ert)
            onrm.append(t)

        for i in range(2):
            yps = gpt([128, C])
            for m in range(3):
                nc.tensor.matmul(
                    out=yps,
                    lhsT=onrm[m][:, 128 * i:128 * i + 128],
                    rhs=wo[:, m, :],
                    start=(m == 0), stop=(m == 2))
            ysb = sb.tile([128, C], f32, tag=f"ysb{i}", name="ysb")
            nc.vector.tensor_copy(out=ysb, in_=yps)
            # contiguous DRAM write; the host gather undoes the node order
            # (y-tile i covers g01 == i, rows (pair, i64))
            nc.sync.dma_start(
                out=y_d.rearrange("(i r) e -> i r e", i=2)[i],
                in_=ysb)

    nc.compile()
    return nc


def _get_nc():
    if "nc" not in _CACHE:
        _CACHE["nc"] = _build_nc()
    return _CACHE["nc"]


def make_in_maps(inputs):
    x = np.asarray(inputs["x"], np.float32)
    pos = np.asarray(inputs["pos"], np.float32)
    freqs = np.asarray(inputs["rope_freqs"], np.float32)
    prep = _host_prep(np.asarray(inputs["Wq"], np.float32),
                      np.asarray(inputs["Wk"], np.float32),
                      np.asarray(inputs["Wv"], np.float32),
                      np.asarray(inputs["Wo"], np.float32),
                      freqs)
    in_maps = []
    import ml_dtypes
    for c in range(NCORES):
        sl = slice(c * NPC, (c + 1) * NPC)
        m = dict(prep)
        xs = x[sl].T                                # [384, 256]
        m["xT"] = np.ascontiguousarray(
            xs.reshape(3, 128, NPC).transpose(1, 0, 2).reshape(128, 3 * NPC)
            .astype(ml_dtypes.bfloat16))
        m["cs"] = _rope_cache(pos[sl], freqs)
        in_maps.append(m)
    return in_maps


def gather(res):
    """Assemble the full [N, C] output; undoes the per-core (g01, pair, i64)
    node-column order the kernel keeps for contiguous DRAM writes."""
    outs = []
    for c in range(NCORES):
        yr = np.asarray(res.results[c]["y"], np.float32)   # [256, 384] raw
        outs.append(yr.reshape(2, 2, 64, C).transpose(1, 0, 2, 3).reshape(NPC, C))
    return np.concatenate(outs, axis=0)


def kernel(**inputs):
    from concourse.bass_utils import run_bass_kernel_spmd

    in_maps = make_in_maps(inputs)

    nc = _get_nc()
    res = run_bass_kernel_spmd(nc, in_maps, core_ids=list(range(NCORES)))
    return gather(res)


# revision 48
# speedup vs baseline: 1.9224x; 1.1153x over previous
"""PlatonicConv (graph-mode attention) Trainium2 Bass kernel.

Math (per graph of 64 fully-connected nodes, 24 group-heads of dim 16):
  q/k/v = x @ W; RoPE(q, k) from pos; S = q.k^T/4; softmax over dst;
  out = A @ v; y = out @ Wo.  32 graphs -> data-parallel over 8 cores.

Layout choices (per core: 4 graphs, 256 nodes):
  * Attention side lives transposed ([feature, node]); x is transposed on host.
  * RoPE cos/sin caches are host-precomputed patterns [128, 256]; the rotation
    partner-swap + sign is folded into a second spread matrix (esp2b), so the
    pair-swapped projections (Wqp/Wkp) are not needed: only one projection per
    q/k slab, then  spread = esp2a^T (qt*cos) + esp2b^T (qt*sin)  accumulated
    in PSUM (this also absorbs the rope add).
  * Heads are "spread" to 32-aligned partition slots so score matmuls pack
    4-way into the PE array via tile_position row groups.
  * Softmax is max-free (scores are O(1) by construction); the denominator
    comes for free as a 17th row of each AV matmul via an interleaved
    ones-column in the V weights.
  * The spread->compact "unspread" after AV is a single selection matmul per
    head-quad (rows 0-3 = denominators, 4-67 = compact out); og moves by one
    DMA per quad and the denominators are reciprocal'd straight out of PSUM
    by the scalar engine.
"""

import numpy as np

G = 12
H = 2
D = 16
GH = 24          # G * H group-heads
C = 384          # in/emb/out channels
NG = 32          # graphs
NPG = 64         # nodes per graph
N = NG * NPG
NCORES = 8
GPC = NG // NCORES   # graphs per core = 4
NPC = GPC * NPG      # nodes per core = 256
VW = 17              # V block width (16 + ones col)
CAUG = GH * VW       # 408
CSTW = 1116          # packed consts width

_CACHE = {}


def _host_prep(Wq, Wk, Wv, Wo, rope_freqs):
    import ml_dtypes
    f32 = np.float32

    # esp2a: compact row (within 64-block) -> 32-aligned spread slot; stacked
    # twice so odd 64-row slabs can use base partition 64
    esp2a = np.zeros((128, 128), f32)
    for k in range(64):
        m = 32 * (k // 16) + (k % 16)
        esp2a[k, m] = 1.0
        esp2a[64 + k, m] = 1.0
    # esp2b = P^T esp2a: P = rope pair-swap (d even<->odd) with sign(-1 on even)
    p64 = np.arange(64) ^ 1
    s64 = np.where(np.arange(64) % 2 == 0, -1.0, 1.0).astype(f32)
    esp2b = np.zeros((128, 128), f32)
    for c in range(64):
        r = p64[c]
        esp2b[c, :] = s64[r] * esp2a[r, :]
        esp2b[64 + c, :] = s64[r] * esp2a[64 + r, :]

    # unspread selection, parity-matched so every downstream engine move is
    # partition-shift-free:
    #   even quads: og -> out rows 0-63, den -> rows 64-67
    #   odd quads:  og -> out rows 64-127, den -> rows 0-3
    selE = np.zeros((128, 68), f32)
    selO = np.zeros((128, 128), f32)
    for a in range(4):
        selE[32 * a + 16, 64 + a] = 1.0
        selO[32 * a + 16, a] = 1.0
        for d in range(16):
            selE[32 * a + d, 16 * a + d] = 1.0
            selO[32 * a + d, 64 + 16 * a + d] = 1.0

    # e68: rden2 rows (64-67 = even-quad dens -> out rows 0-63; 0-3 = odd-quad
    # dens -> out rows 64-127), broadcast to 16 consecutive emb rows each
    e68 = np.zeros((68, 128), f32)
    for i in range(64):
        e68[64 + i // 16, i] = 1.0
        e68[i // 16, 64 + i] = 1.0

    # packed consts [128, 1052]
    cst = np.zeros((128, CSTW), ml_dtypes.bfloat16)
    cst[:, 0:128] = esp2a
    cst[:, 128:256] = esp2b
    cst[:, 256:324] = selE
    cst[:, 324:452] = selO
    cst[0:68, 452:580] = e68
    cst[0, 580:708] = 1.0                        # onesrow
    cst[0, 708 + VW * np.arange(GH) + 16] = 1.0  # vseed (cols 708:1116)

    # V interleaved with a ones column per head: block j = [Wv head j | 0]
    Wvil = np.zeros((C, CAUG), f32)
    for j in range(GH):
        Wvil[:, VW * j:VW * j + 16] = Wv[:, 16 * j:16 * j + 16]

    bf16 = ml_dtypes.bfloat16

    def pack(w):
        # [384, cols] -> [128, 3*cols]: col block s = w[128 s : 128 s + 128]
        cols = w.shape[1]
        return np.ascontiguousarray(
            w.reshape(3, 128, cols).transpose(1, 0, 2).reshape(128, 3 * cols)
            .astype(bf16))

    return dict(
        wq=pack(Wq), wk=pack(Wk), wvil=pack(Wvil), wo=pack(Wo), cst=cst,
        e68=np.ascontiguousarray(e68),
    )


def _rope_cache(pos, rope_freqs):
    # cos/sin patterns [128, 256]: row r (mod 64) = 16 m + d -> head h = m%2,
    # freq index d//2; two stacked 64-row copies
    f32 = np.float32
    theta = np.einsum('ns,shf->nhf', pos.astype(f32), rope_freqs.astype(f32))
    r = np.arange(64)
    h = (r // 16) % 2
    f = (r % 16) // 2
    cpat = np.cos(theta[:, h, f]).T.astype(f32)   # [64, 256]
    spat = np.sin(theta[:, h, f]).T.astype(f32)
    cs = np.empty((128, 2 * NPC), f32)
    cs[0:64, 0:NPC] = cpat
    cs[64:128, 0:NPC] = cpat
    cs[0:64, NPC:] = spat
    cs[64:128, NPC:] = spat
    return cs


def _build_nc():
    import concourse.bacc as bacc
    import concourse.tile as tile
    import concourse.mybir as mybir
    from contextlib import ExitStack

    f32 = mybir.dt.float32
    fmm = mybir.dt.float32r
    fb = mybir.dt.bfloat16
    AF = mybir.ActivationFunctionType

    nc = bacc.Bacc("TRN2", target_bir_lowering=False)

    xT_d = nc.dram_tensor("xT", [128, 3 * NPC], fb, kind="ExternalInput")
    cs_d = nc.dram_tensor("cs", [128, 2 * NPC], f32, kind="ExternalInput")
    wq_d = nc.dram_tensor("wq", [128, 3 * C], fb, kind="ExternalInput")
    wk_d = nc.dram_tensor("wk", [128, 3 * C], fb, kind="ExternalInput")
    wvil_d = nc.dram_tensor("wvil", [128, 3 * CAUG], fb, kind="ExternalInput")
    wo_d = nc.dram_tensor("wo", [128, 3 * C], fb, kind="ExternalInput")
    cst_d = nc.dram_tensor("cst", [128, CSTW], fb, kind="ExternalInput")
    e68_d = nc.dram_tensor("e68", [68, 128], f32, kind="ExternalInput")
    y_d = nc.dram_tensor("y", [NPC, C], f32, kind="ExternalOutput")

    ctx = ExitStack()
    with tile.TileContext(nc) as tc, ctx:
        consts = ctx.enter_context(tc.tile_pool(name="consts", bufs=1))
        sb = ctx.enter_context(tc.tile_pool(name="sbuf", bufs=1))
        # general psum: shared tag -> recycled 1-bank slots
        ps_gp = ctx.enter_context(tc.tile_pool(name="ps_gp", bufs=2, space="PSUM"))
        ps_att = ctx.enter_context(tc.tile_pool(name="ps_att", bufs=1, space="PSUM"))
        ps_av = ctx.enter_context(tc.tile_pool(name="ps_av", bufs=1, space="PSUM"))

        def gpt(shape):
            return ps_gp.tile(shape, f32, tag="pp", name="pp")

        # ---- input DMAs, split across the two queues; weights per-slab so
        # the first projection only waits on xT + one slab ----
        xT = consts.tile([128, 3, NPC], fb, tag="xT")
        nc.sync.dma_start(out=xT, in_=xT_d.rearrange("p (s e) -> p s e", s=3))
        cs = consts.tile([128, 2, NPC], f32, tag="cs")
        nc.scalar.dma_start(out=cs, in_=cs_d.rearrange("p (s e) -> p s e", s=2))

        def load_w(dram, cols, tag, eng):
            t = consts.tile([128, 3, cols], fb, tag=tag)
            dv = dram.rearrange("p (s e) -> p s e", s=3)
            for s in range(3):
                eng.dma_start(out=t[:, s, :], in_=dv[:, s, :])
            return t
        wq = load_w(wq_d, C, "wq", nc.sync)
        cst = consts.tile([128, CSTW], fb, tag="cst")
        nc.scalar.dma_start(out=cst, in_=cst_d[:])
        wk = load_w(wk_d, C, "wk", nc.scalar)
        wvil = load_w(wvil_d, CAUG, "wvil", nc.sync)
        wo = load_w(wo_d, C, "wo", nc.scalar)
        e68 = consts.tile([68, 128], f32, tag="e68")
        nc.scalar.dma_start(out=e68, in_=e68_d[:])

        cosf = cs[:, 0, :]
        sinf = cs[:, 1, :]
        esp2a = cst[:, 0:128]
        esp2b = cst[:, 128:256]
        selE = cst[:, 256:324]
        selO = cst[:, 324:452]
        onesrow = cst[0:1, 580:708]
        vseed = cst[0:1, 708:708 + CAUG]

        # ---- projections (transposed) + RoPE + spread, per 128-row m-slab.
        # spread = esp2a^T (qt*cos) + esp2b^T (qt*sin), accumulated in psum.
        # q/k interleaved per slab so the PE has projection work to do while
        # the DVE muls feed the spread matmuls; psum->sbuf copies run on the
        # scalar engine to keep the DVE stream short.
        qsp, ksp = [], []
        for m in range(3):
            ab = {}
            for tag, w in (("q", wq), ("k", wk)):
                ps = gpt([128, NPC])
                for k in range(3):
                    nc.tensor.matmul(
                        out=ps,
                        lhsT=w[:, k, 128 * m:128 * m + 128],
                        rhs=xT[:, k, :],
                        start=(k == 0), stop=(k == 2))
                a = sb.tile([128, NPC], fb, tag=f"ra{tag}{m}")
                b = sb.tile([128, NPC], fb, tag=f"rb{tag}{m}")
                nc.vector.tensor_mul(out=a, in0=ps, in1=cosf)
                nc.vector.tensor_mul(out=b, in0=ps, in1=sinf)
                ab[tag] = (a, b)
            for tag, lst in (("q", qsp), ("k", ksp)):
                a, b = ab[tag]
                for half in range(2):
                    hs = slice(64 * half, 64 * half + 64)
                    sp = gpt([128, NPC])
                    nc.tensor.matmul(out=sp, lhsT=esp2a[hs, :], rhs=a[hs, :],
                                     start=True, stop=False)
                    nc.tensor.matmul(out=sp, lhsT=esp2b[hs, :], rhs=b[hs, :],
                                     start=False, stop=True)
                    # bf16 (not f32r): score matmuls use tile_position dst
                    # offsets that are invalid for f32r operands
                    t = sb.tile([128, NPC], fb, tag=f"sps{tag}{2 * m + half}")
                    nc.scalar.activation(out=t, in_=sp, func=AF.Copy)
                    lst.append(t)

        # ---- V_aug [256, 408] untransposed (+ ones cols via K=1 matmul) ----
        vau = []
        for i in range(2):
            ps = gpt([128, CAUG])
            for k in range(3):
                nc.tensor.matmul(
                    out=ps,
                    lhsT=xT[:, k, 128 * i:128 * i + 128],
                    rhs=wvil[:, k, :],
                    start=(k == 0), stop=False)
            nc.tensor.matmul(
                out=ps, lhsT=onesrow, rhs=vseed,
                start=False, stop=True)
            t = sb.tile([128, CAUG], fb, tag=f"vau{i}")
            nc.vector.tensor_copy(out=t, in_=ps)
            vau.append(t)

        # ---- scores S^T + exp, per graph-pair.
        # Concurrent row-tiled matmuls MUST land in distinct PSUM banks:
        # head gh -> bank gh%4 (512-col block), col 64*(gh//4), rows 64*g01.
        def scol(gh):
            return 512 * (gh % 4) + 64 * (gh // 4)

        expst = []
        for pair in range(2):
            stp = ps_att.tile([128, 4 * 512], f32, tag="stps")
            for gh in range(GH):
                tilei, slot = divmod(gh, 4)
                lo = 32 * slot
                for g01 in range(2):
                    g = 2 * pair + g01
                    nc.tensor.matmul(
                        out=stp[64 * g01:64 * g01 + 64, scol(gh):scol(gh) + 64],
                        lhsT=ksp[tilei][lo:lo + 16, 64 * g:64 * g + 64],
                        rhs=qsp[tilei][lo:lo + 16, 64 * g:64 * g + 64],
                        start=True, stop=True,
                        tile_position=(lo, 64 * g01))
            et = sb.tile([128, 4 * 512], fb, tag=f"expst{pair}")
            for b in range(4):
                nc.scalar.activation(
                    out=et[:, 512 * b:512 * b + 384],
                    in_=stp[:, 512 * b:512 * b + 384],
                    func=AF.Exp, scale=0.25)
            expst.append(et)

        # ---- AV (+den row): per (quad, parity) [128, 128] psum tiles so the
        # two concurrent row groups (graph parities) use distinct banks;
        # head slot 32*(gh%4) rows, col 64*(g//2).
        # Then unspread via one sel matmul: rows 0-3 = den, 4-67 = compact out.
        avsb = sb.tile([128, 6 * 256], fb, tag="avsb")
        og = [sb.tile([128, NPC], f32, tag=f"og{m}", name="og") for m in range(3)]
        # lden: ln(den); quad pair p -> cols 256p; even quad dens at rows
        # 64-67, odd at rows 0-3 (parity-matched to the Ln source partitions).
        # Rows 4-63 are never written but read by the e68 matmul: zero once.
        # (A DVE reciprocal costs ~1.7us flat, so normalization goes through
        # scalar Ln -> e68 broadcast matmul -> scalar Exp(scale=-1) instead.)
        lden = sb.tile([68, 3 * NPC], f32, tag="lden")
        nc.vector.memset(lden, 0.0)
        # persistent per-parity AV psum tiles with 4 column regions (qd % 4)
        # -> 4-deep quad pipelining within 2 banks, no recycle stalls
        avt = [ps_av.tile([128, 512], f32, tag=f"av{g01}", name="av")
               for g01 in range(2)]
        nc.vector.memset(avt[0], 0.0)
        nc.vector.memset(avt[1], 0.0)
        for qd in range(6):
            reg = 128 * (qd % 4)
            for g in range(GPC):
                pair, g01 = divmod(g, 2)
                lo = 64 * g01
                for a in range(4):
                    gh = 4 * qd + a
                    nc.tensor.matmul(
                        out=avt[g01][32 * a:32 * a + VW,
                                     reg + 64 * (g // 2):reg + 64 * (g // 2) + 64],
                        lhsT=vau[pair][lo:lo + 64, VW * gh:VW * gh + VW],
                        rhs=expst[pair][lo:lo + 64, scol(gh):scol(gh) + 64],
                        start=True, stop=True,
                        tile_position=(lo, 32 * a))
            cq = slice(256 * qd, 256 * qd + 256)
            for g01 in range(2):
                nc.vector.tensor_copy(
                    out=avsb[:, 256 * qd + 128 * g01:256 * qd + 128 * g01 + 128],
                    in_=avt[g01][:, reg:reg + 128])
            odd = qd % 2
            ups = gpt([128, NPC])
            nc.tensor.matmul(out=ups[0:68, :] if not odd else ups,
                             lhsT=selO if odd else selE, rhs=avsb[:, cq],
                             start=True, stop=True)
            ohs = slice(64 * odd, 64 * odd + 64)
            dhs = slice(64 - 64 * odd, 68 - 64 * odd)
            rp = slice(NPC * (qd // 2), NPC * (qd // 2) + NPC)
            nc.scalar.activation(out=lden[dhs, rp], in_=ups[dhs, :],
                                 func=AF.Ln)
            # node columns stay in (g01, pair, i) order (host gather undoes)
            nc.vector.tensor_copy(out=og[qd // 2][ohs, :], in_=ups[ohs, :])

        # ---- normalize + y = O_norm @ Wo ----
        onrm = []
        for m in range(3):
            rt = gpt([128, NPC])
            rp = slice(NPC * m, NPC * m + NPC)
            nc.tensor.matmul(out=rt, lhsT=e68, rhs=lden[:, rp],
                             start=True, stop=True)
            ert = sb.tile([128, NPC], f32, tag=f"ert{m}")
            nc.scalar.activation(out=ert, in_=rt, func=AF.Exp, scale=-1.0)
            t = sb.tile([128, NPC], fb, tag=f"onrm{m}")
            nc.vector.tensor_mul(out=t, in0=og[m], in1=ert)
            onrm.append(t)

        for i in range(2):
            yps = gpt([128, C])
            for m in range(3):
                nc.tensor.matmul(
                    out=yps,
                    lhsT=onrm[m][:, 128 * i:128 * i + 128],
                    rhs=wo[:, m, :],
                    start=(m == 0), stop=(m == 2))
            ysb = sb.tile([128, C], f32, tag=f"ysb{i}", name="ysb")
            nc.vector.tensor_copy(out=ysb, in_=yps)
            # contiguous DRAM write; the host gather undoes the node order
            # (y-tile i covers g01 == i, rows (pair, i64))
            nc.sync.dma_start(
                out=y_d.rearrange("(i r) e -> i r e", i=2)[i],
                in_=ysb)

    nc.compile()
    return nc


def _get_nc():
    if "nc" not in _CACHE:
        _CACHE["nc"] = _build_nc()
    return _CACHE["nc"]


def make_in_maps(inputs):
    x = np.asarray(inputs["x"], np.float32)
    pos = np.asarray(inputs["pos"], np.float32)
    freqs = np.asarray(inputs["rope_freqs"], np.float32)
    prep = _host_prep(np.asarray(inputs["Wq"], np.float32),
                      np.asarray(inputs["Wk"], np.float32),
                      np.asarray(inputs["Wv"], np.float32),
                      np.asarray(inputs["Wo"], np.float32),
                      freqs)
    in_maps = []
    import ml_dtypes
    for c in range(NCORES):
        sl = slice(c * NPC, (c + 1) * NPC)
        m = dict(prep)
        xs = x[sl].T                                # [384, 256]
        m["xT"] = np.ascontiguousarray(
            xs.reshape(3, 128, NPC).transpose(1, 0, 2).reshape(128, 3 * NPC)
            .astype(ml_dtypes.bfloat16))
        m["cs"] = _rope_cache(pos[sl], freqs)
        in_maps.append(m)
    return in_maps


def gather(res):
    """Assemble the full [N, C] output; undoes the per-core (g01, pair, i64)
    node-column order the kernel keeps for contiguous DRAM writes."""
    outs = []
    for c in range(NCORES):
        yr = np.asarray(res.results[c]["y"], np.float32)   # [256, 384] raw
        outs.append(yr.reshape(2, 2, 64, C).transpose(1, 0, 2, 3).reshape(NPC, C))
    return np.concatenate(outs, axis=0)


def kernel(**inputs):
    from concourse.bass_utils import run_bass_kernel_spmd

    in_maps = make_in_maps(inputs)

    nc = _get_nc()
    res = run_bass_kernel_spmd(nc, in_maps, core_ids=list(range(NCORES)))
    return gather(res)


# revision 59
# speedup vs baseline: 1.9421x; 1.0102x over previous
"""PlatonicConv (graph-mode attention) Trainium2 Bass kernel.

Math (per graph of 64 fully-connected nodes, 24 group-heads of dim 16):
  q/k/v = x @ W; RoPE(q, k) from pos; S = q.k^T/4; softmax over dst;
  out = A @ v; y = out @ Wo.  32 graphs -> data-parallel over 8 cores.

Layout choices (per core: 4 graphs, 256 nodes):
  * Attention side lives transposed ([feature, node]); x is transposed on host.
  * RoPE cos/sin caches are host-precomputed patterns [128, 256]; the rotation
    partner-swap + sign is folded into a second spread matrix (esp2b), so the
    pair-swapped projections (Wqp/Wkp) are not needed: only one projection per
    q/k slab, then  spread = esp2a^T (qt*cos) + esp2b^T (qt*sin)  accumulated
    in PSUM (this also absorbs the rope add).
  * Heads are "spread" to 32-aligned partition slots so score matmuls pack
    4-way into the PE array via tile_position row groups.
  * Softmax is max-free (scores are O(1) by construction); the denominator
    comes for free as a 17th row of each AV matmul via an interleaved
    ones-column in the V weights.
  * The spread->compact "unspread" after AV is a single selection matmul per
    head-quad (rows 0-3 = denominators, 4-67 = compact out); og moves by one
    DMA per quad and the denominators are reciprocal'd straight out of PSUM
    by the scalar engine.
"""

import numpy as np

G = 12
H = 2
D = 16
GH = 24          # G * H group-heads
C = 384          # in/emb/out channels
NG = 32          # graphs
NPG = 64         # nodes per graph
N = NG * NPG
NCORES = 8
GPC = NG // NCORES   # graphs per core = 4
NPC = GPC * NPG      # nodes per core = 256
VW = 17              # V block width (16 + ones col)
CAUG = GH * VW       # 408
CSTW = 1116          # packed consts width

_CACHE = {}


def _host_prep(Wq, Wk, Wv, Wo, rope_freqs):
    import ml_dtypes
    f32 = np.float32

    # esp2a: compact row (within 64-block) -> 32-aligned spread slot; stacked
    # twice so odd 64-row slabs can use base partition 64
    esp2a = np.zeros((128, 128), f32)
    for k in range(64):
        m = 32 * (k // 16) + (k % 16)
        esp2a[k, m] = 1.0
        esp2a[64 + k, m] = 1.0
    # esp2b = P^T esp2a: P = rope pair-swap (d even<->odd) with sign(-1 on even)
    p64 = np.arange(64) ^ 1
    s64 = np.where(np.arange(64) % 2 == 0, -1.0, 1.0).astype(f32)
    esp2b = np.zeros((128, 128), f32)
    for c in range(64):
        r = p64[c]
        esp2b[c, :] = s64[r] * esp2a[r, :]
        esp2b[64 + c, :] = s64[r] * esp2a[64 + r, :]

    # unspread selection, parity-matched so every downstream engine move is
    # partition-shift-free:
    #   even quads: og -> out rows 0-63, den -> rows 64-67
    #   odd quads:  og -> out rows 64-127, den -> rows 0-3
    selE = np.zeros((128, 68), f32)
    selO = np.zeros((128, 128), f32)
    for a in range(4):
        selE[32 * a + 16, 64 + a] = 1.0
        selO[32 * a + 16, a] = 1.0
        for d in range(16):
            selE[32 * a + d, 16 * a + d] = 1.0
            selO[32 * a + d, 64 + 16 * a + d] = 1.0

    # e68: rden2 rows (64-67 = even-quad dens -> out rows 0-63; 0-3 = odd-quad
    # dens -> out rows 64-127), broadcast to 16 consecutive emb rows each
    e68 = np.zeros((68, 128), f32)
    for i in range(64):
        e68[64 + i // 16, i] = 1.0
        e68[i // 16, 64 + i] = 1.0

    # packed consts [128, 1052]
    cst = np.zeros((128, CSTW), ml_dtypes.bfloat16)
    cst[:, 0:128] = esp2a
    cst[:, 128:256] = esp2b
    cst[:, 256:324] = selE
    cst[:, 324:452] = selO
    cst[0:68, 452:580] = e68
    cst[0, 580:708] = 1.0                        # onesrow
    cst[0, 708 + VW * np.arange(GH) + 16] = 1.0  # vseed (cols 708:1116)

    # V interleaved with a ones column per head: block j = [Wv head j | 0]
    Wvil = np.zeros((C, CAUG), f32)
    for j in range(GH):
        Wvil[:, VW * j:VW * j + 16] = Wv[:, 16 * j:16 * j + 16]

    bf16 = ml_dtypes.bfloat16

    def pack(w):
        # [384, cols] -> [128, 3*cols]: col block s = w[128 s : 128 s + 128]
        cols = w.shape[1]
        return np.ascontiguousarray(
            w.reshape(3, 128, cols).transpose(1, 0, 2).reshape(128, 3 * cols)
            .astype(bf16))

    wko = np.concatenate([pack(Wk).reshape(128, 3, C),
                          pack(Wo).reshape(128, 3, C)], axis=2)
    return dict(
        wko=np.ascontiguousarray(wko.reshape(128, 3 * 2 * C)),
        wvil=pack(Wvil), cst=cst,
        e68=np.ascontiguousarray(e68),
        _wq=pack(Wq).reshape(128, 3, C),
    )


def _rope_cache(pos, rope_freqs):
    # cos/sin patterns [128, 256]: row r (mod 64) = 16 m + d -> head h = m%2,
    # freq index d//2; two stacked 64-row copies
    f32 = np.float32
    theta = np.einsum('ns,shf->nhf', pos.astype(f32), rope_freqs.astype(f32))
    r = np.arange(64)
    h = (r // 16) % 2
    f = (r % 16) // 2
    cpat = np.cos(theta[:, h, f]).T.astype(f32)   # [64, 256]
    spat = np.sin(theta[:, h, f]).T.astype(f32)
    cs = np.empty((128, 2 * NPC), f32)
    cs[0:64, 0:NPC] = cpat
    cs[64:128, 0:NPC] = cpat
    cs[0:64, NPC:] = spat
    cs[64:128, NPC:] = spat
    return cs


def _build_nc():
    import concourse.bacc as bacc
    import concourse.tile as tile
    import concourse.mybir as mybir
    from contextlib import ExitStack

    f32 = mybir.dt.float32
    fmm = mybir.dt.float32r
    fb = mybir.dt.bfloat16
    AF = mybir.ActivationFunctionType

    nc = bacc.Bacc("TRN2", target_bir_lowering=False)

    # xtw: per-slab [xT_s | Wq_s] so one DMA feeds the first projections;
    # wko: per-slab [Wk_s | Wo_s]
    xtw_d = nc.dram_tensor("xtw", [128, 3 * (NPC + C)], fb, kind="ExternalInput")
    cs_d = nc.dram_tensor("cs", [128, 2 * NPC], f32, kind="ExternalInput")
    wko_d = nc.dram_tensor("wko", [128, 3 * 2 * C], fb, kind="ExternalInput")
    wvil_d = nc.dram_tensor("wvil", [128, 3 * CAUG], fb, kind="ExternalInput")
    cst_d = nc.dram_tensor("cst", [128, CSTW], fb, kind="ExternalInput")
    e68_d = nc.dram_tensor("e68", [68, 128], f32, kind="ExternalInput")
    y_d = nc.dram_tensor("y", [NPC, C], f32, kind="ExternalOutput")

    ctx = ExitStack()
    with tile.TileContext(nc) as tc, ctx:
        consts = ctx.enter_context(tc.tile_pool(name="consts", bufs=1))
        sb = ctx.enter_context(tc.tile_pool(name="sbuf", bufs=1))
        # general psum: shared tag -> recycled 1-bank slots
        ps_gp = ctx.enter_context(tc.tile_pool(name="ps_gp", bufs=2, space="PSUM"))
        ps_att = ctx.enter_context(tc.tile_pool(name="ps_att", bufs=1, space="PSUM"))
        ps_av = ctx.enter_context(tc.tile_pool(name="ps_av", bufs=1, space="PSUM"))

        def gpt(shape):
            return ps_gp.tile(shape, f32, tag="pp", name="pp")

        # ---- input DMAs: few large single-descriptor transfers, ordered by
        # first use, split across the two queues ----
        cs = consts.tile([128, 2, NPC], f32, tag="cs")
        nc.sync.dma_start(out=cs, in_=cs_d.rearrange("p (s e) -> p s e", s=2))
        xtw = consts.tile([128, 3, NPC + C], fb, tag="xtw")
        nc.sync.dma_start(out=xtw,
                          in_=xtw_d.rearrange("p (s e) -> p s e", s=3))
        wko = consts.tile([128, 3, 2 * C], fb, tag="wko")
        nc.scalar.dma_start(out=wko,
                            in_=wko_d.rearrange("p (s e) -> p s e", s=3))
        cst = consts.tile([128, CSTW], fb, tag="cst")
        nc.scalar.dma_start(out=cst, in_=cst_d[:])
        wvil = consts.tile([128, 3, CAUG], fb, tag="wvil")
        nc.sync.dma_start(out=wvil,
                          in_=wvil_d.rearrange("p (s e) -> p s e", s=3))
        e68 = consts.tile([68, 128], f32, tag="e68")
        nc.scalar.dma_start(out=e68, in_=e68_d[:])

        cosf = cs[:, 0, :]
        sinf = cs[:, 1, :]
        esp2a = cst[:, 0:128]
        esp2b = cst[:, 128:256]
        selE = cst[:, 256:324]
        selO = cst[:, 324:452]
        onesrow = cst[0:1, 580:708]
        vseed = cst[0:1, 708:708 + CAUG]

        # ---- projections (transposed) + RoPE + spread, per 128-row m-slab.
        # spread = esp2a^T (qt*cos) + esp2b^T (qt*sin), accumulated in psum.
        # q/k interleaved per slab so the PE has projection work to do while
        # the DVE muls feed the spread matmuls; both 64-row halves land in
        # one single-bank [128, 512] psum tile -> one scalar copy each, and
        # the PE never waits on a copy for a psum slot.
        # Pair-0 scores for slab m-1 ride along after slab m.
        def proj(w, woff, m):
            ps = gpt([128, NPC])
            for k in range(3):
                nc.tensor.matmul(
                    out=ps,
                    lhsT=w[:, k, woff + 128 * m:woff + 128 * m + 128],
                    rhs=xtw[:, k, 0:NPC],
                    start=(k == 0), stop=(k == 2))
            return ps

        # qsp/ksp: per slab m one [128, 512] tile; tilei 2m+half -> col half
        def scol(gh):
            return 512 * (gh % 4) + 64 * (gh // 4)

        def sview(lst, tilei):
            return lst[tilei // 2][:, 256 * (tilei % 2):256 * (tilei % 2) + 256]

        def emit_scores(stp, pair, tiles):
            # Concurrent row-tiled matmuls MUST land in distinct PSUM banks:
            # head gh -> bank gh%4 (512-col block), col 64*(gh//4), rows 64*g01
            for tilei in tiles:
                for slot in range(4):
                    gh = 4 * tilei + slot
                    lo = 32 * slot
                    for g01 in range(2):
                        g = 2 * pair + g01
                        nc.tensor.matmul(
                            out=stp[64 * g01:64 * g01 + 64,
                                    scol(gh):scol(gh) + 64],
                            lhsT=sview(ksp, tilei)[lo:lo + 16, 64 * g:64 * g + 64],
                            rhs=sview(qsp, tilei)[lo:lo + 16, 64 * g:64 * g + 64],
                            start=True, stop=True,
                            tile_position=(lo, 64 * g01))

        qsp, ksp = [], []
        vau = []
        for m in range(3):
            ab = {}
            for tag, woff in (("q", NPC), ("k", 0)):
                ps = proj(xtw if tag == "q" else wko, woff, m)
                a = sb.tile([128, NPC], fb, tag=f"ra{tag}{m}")
                b = sb.tile([128, NPC], fb, tag=f"rb{tag}{m}")
                nc.vector.tensor_mul(out=a, in0=ps, in1=cosf)
                nc.vector.tensor_mul(out=b, in0=ps, in1=sinf)
                ab[tag] = (a, b)
            for tag, lst in (("q", qsp), ("k", ksp)):
                a, b = ab[tag]
                # bf16 (not f32r): score matmuls use tile_position dst
                # offsets that are invalid for f32r operands
                t = sb.tile([128, 2 * NPC], fb, tag=f"sps{tag}{m}")
                for half in range(2):
                    hs = slice(64 * half, 64 * half + 64)
                    csl = slice(NPC * half, NPC * half + NPC)
                    sp = gpt([128, NPC])
                    nc.tensor.matmul(out=sp, lhsT=esp2a[hs, :],
                                     rhs=a[hs, :], start=True, stop=False)
                    nc.tensor.matmul(out=sp, lhsT=esp2b[hs, :],
                                     rhs=b[hs, :], start=False, stop=True)
                    nc.scalar.activation(out=t[:, csl], in_=sp, func=AF.Copy)
                lst.append(t)
            if m == 0:
                # V_aug [256, 408] untransposed (+ ones cols via K=1 matmul);
                # fills the PE while slab-0 copies drain
                for i in range(2):
                    ps = gpt([128, CAUG])
                    for k in range(3):
                        nc.tensor.matmul(
                            out=ps,
                            lhsT=xtw[:, k, 128 * i:128 * i + 128],
                            rhs=wvil[:, k, :],
                            start=(k == 0), stop=False)
                    nc.tensor.matmul(
                        out=ps, lhsT=onesrow, rhs=vseed,
                        start=False, stop=True)
                    t = sb.tile([128, CAUG], fb, tag=f"vau{i}")
                    nc.vector.tensor_copy(out=t, in_=ps)
                    vau.append(t)
        # ---- scores + exp per graph-pair ----
        expst = []
        for pair in range(2):
            stp = ps_att.tile([128, 4 * 512], f32, tag="stps")
            emit_scores(stp, pair, range(6))
            et = sb.tile([128, 4 * 512], fb, tag=f"expst{pair}")
            for b in range(4):
                nc.scalar.activation(
                    out=et[:, 512 * b:512 * b + 384],
                    in_=stp[:, 512 * b:512 * b + 384],
                    func=AF.Exp, scale=0.25)
            expst.append(et)

        # ---- AV (+den row): per (quad, parity) [128, 128] psum tiles so the
        # two concurrent row groups (graph parities) use distinct banks;
        # head slot 32*(gh%4) rows, col 64*(g//2).
        # Then unspread via one sel matmul: rows 0-3 = den, 4-67 = compact out.
        avsb = sb.tile([128, 6 * 256], fb, tag="avsb")
        og = [sb.tile([128, NPC], f32, tag=f"og{m}", name="og") for m in range(3)]
        # lden: ln(den); quad pair p -> cols 256p; even quad dens at rows
        # 64-67, odd at rows 0-3 (parity-matched to the Ln source partitions).
        # Rows 4-63 are never written but read by the e68 matmul: zero once.
        # (A DVE reciprocal costs ~1.7us flat, so normalization goes through
        # scalar Ln -> e68 broadcast matmul -> scalar Exp(scale=-1) instead.)
        lden = sb.tile([68, 3 * NPC], f32, tag="lden")
        nc.vector.memset(lden, 0.0)
        # persistent per-parity AV psum tiles with 4 column regions (qd % 4)
        # -> 4-deep quad pipelining within 2 banks, no recycle stalls
        avt = [ps_av.tile([128, 512], f32, tag=f"av{g01}", name="av")
               for g01 in range(2)]
        nc.vector.memset(avt[0], 0.0)
        nc.vector.memset(avt[1], 0.0)
        for qd in range(6):
            reg = 128 * (qd % 4)
            for g in range(GPC):
                pair, g01 = divmod(g, 2)
                lo = 64 * g01
                for a in range(4):
                    gh = 4 * qd + a
                    nc.tensor.matmul(
                        out=avt[g01][32 * a:32 * a + VW,
                                     reg + 64 * (g // 2):reg + 64 * (g // 2) + 64],
                        lhsT=vau[pair][lo:lo + 64, VW * gh:VW * gh + VW],
                        rhs=expst[pair][lo:lo + 64, scol(gh):scol(gh) + 64],
                        start=True, stop=True,
                        tile_position=(lo, 32 * a))
            cq = slice(256 * qd, 256 * qd + 256)
            for g01 in range(2):
                nc.vector.tensor_copy(
                    out=avsb[:, 256 * qd + 128 * g01:256 * qd + 128 * g01 + 128],
                    in_=avt[g01][:, reg:reg + 128])
            odd = qd % 2
            ups = gpt([128, NPC])
            nc.tensor.matmul(out=ups[0:68, :] if not odd else ups,
                             lhsT=selO if odd else selE, rhs=avsb[:, cq],
                             start=True, stop=True)
            dhs = slice(64 - 64 * odd, 68 - 64 * odd)
            rp = slice(NPC * (qd // 2), NPC * (qd // 2) + NPC)
            nc.scalar.activation(out=lden[dhs, rp], in_=ups[dhs, :],
                                 func=AF.Ln)
            ohs = slice(64 * odd, 64 * odd + 64)
            # node columns stay in (g01, pair, i) order (host gather undoes)
            nc.vector.tensor_copy(out=og[qd // 2][ohs, :], in_=ups[ohs, :])

        # ---- normalize + y = O_norm @ Wo ----
        onrm = []
        for m in range(3):
            rt = gpt([128, NPC])
            rp = slice(NPC * m, NPC * m + NPC)
            nc.tensor.matmul(out=rt, lhsT=e68, rhs=lden[:, rp],
                             start=True, stop=True)
            ert = sb.tile([128, NPC], f32, tag=f"ert{m}")
            nc.scalar.activation(out=ert, in_=rt, func=AF.Exp, scale=-1.0)
            t = sb.tile([128, NPC], fb, tag=f"onrm{m}")
            nc.vector.tensor_mul(out=t, in0=og[m], in1=ert)
            onrm.append(t)

        for i in range(2):
            yps = gpt([128, C])
            for m in range(3):
                nc.tensor.matmul(
                    out=yps,
                    lhsT=onrm[m][:, 128 * i:128 * i + 128],
                    rhs=wko[:, m, C:2 * C],
                    start=(m == 0), stop=(m == 2))
            ysb = sb.tile([128, C], f32, tag=f"ysb{i}", name="ysb")
            nc.vector.tensor_copy(out=ysb, in_=yps)
            # contiguous DRAM write; the host gather undoes the node order
            # (y-tile i covers g01 == i, rows (pair, i64))
            nc.sync.dma_start(
                out=y_d.rearrange("(i r) e -> i r e", i=2)[i],
                in_=ysb)

    nc.compile()
    return nc


def _get_nc():
    if "nc" not in _CACHE:
        _CACHE["nc"] = _build_nc()
    return _CACHE["nc"]


def make_in_maps(inputs):
    x = np.asarray(inputs["x"], np.float32)
    pos = np.asarray(inputs["pos"], np.float32)
    freqs = np.asarray(inputs["rope_freqs"], np.float32)
    prep = _host_prep(np.asarray(inputs["Wq"], np.float32),
                      np.asarray(inputs["Wk"], np.float32),
                      np.asarray(inputs["Wv"], np.float32),
                      np.asarray(inputs["Wo"], np.float32),
                      freqs)
    in_maps = []
    import ml_dtypes
    wqp = prep.pop("_wq")
    for c in range(NCORES):
        sl = slice(c * NPC, (c + 1) * NPC)
        m = dict(prep)
        xs = x[sl].T                                # [384, 256]
        xTp = (xs.reshape(3, 128, NPC).transpose(1, 0, 2)
               .astype(ml_dtypes.bfloat16))         # [128, 3, 256]
        m["xtw"] = np.ascontiguousarray(
            np.concatenate([xTp, wqp], axis=2).reshape(128, 3 * (NPC + C)))
        m["cs"] = _rope_cache(pos[sl], freqs)
        in_maps.append(m)
    return in_maps


def gather(res):
    """Assemble the full [N, C] output; undoes the per-core (g01, pair, i64)
    node-column order the kernel keeps for contiguous DRAM writes."""
    outs = []
    for c in range(NCORES):
        yr = np.asarray(res.results[c]["y"], np.float32)   # [256, 384] raw
        outs.append(yr.reshape(2, 2, 64, C).transpose(1, 0, 2, 3).reshape(NPC, C))
    return np.concatenate(outs, axis=0)


def kernel(**inputs):
    from concourse.bass_utils import run_bass_kernel_spmd

    in_maps = make_in_maps(inputs)

    nc = _get_nc()
    res = run_bass_kernel_spmd(nc, in_maps, core_ids=list(range(NCORES)))
    return gather(res)
